# revision 1
# baseline (speedup 1.0000x reference)
"""MoE-SIREN (nn_MoE_36146444763329) Trainium2 Bass kernel.

Dense MoE: 8 SIREN experts (1->256->256->256->256->1, sin(30*) activations),
softmax gate over experts, weighted combine. B=2, N=16384 points.

Strategy: data-parallel over the 8 NeuronCores - each core computes all 8
experts for its 4096 points; no collectives. Per core:
  - gate:    z_g = gate_w * x (K=1 matmul) -> exp (ACT, per-partition bias)
             denominator via transposed ones-matmul -> DVE reciprocal
  - layer 0: x broadcast to 128 partitions (ones-matmul), then on GPSIMD:
             zb = x*a + c, k = rne(zb) via magic-add, w = zb - k
  - hidden:  K=256 PE matmuls (2 K-chunks into PSUM), range reduction in ONE
             DVE pass via the ADD_RANGE_WRAP custom op with per-partition
             bias AP (|z+b| <= ~0.77 so a single +-1 period wrap suffices),
             then sin(2*pi*w) on ACT.
  - output:  M=1 matmuls accumulate per-expert rows into an [8,512] PSUM bank
  - combine: (y + bo) * u via scalar_tensor_tensor, per-128-point transposed
             ones-matmul for numerator/denominator, multiply by reciprocal.

All angle math uses units of full turns (weights pre-scaled by omega0/2pi on
the host) so the range reduction is "wrap to [-0.5, 0.5]" and the ACT Sin
gets scale=2*pi, keeping its input inside its valid [-pi, pi] range.
"""
import numpy as np

import concourse.bass as bass
import concourse.mybir as mybir
import concourse.tile as tile
from concourse import bacc
from concourse.bass_utils import run_bass_kernel_spmd
from concourse.dve_ops import ADD_RANGE_WRAP

F32 = mybir.dt.float32
F32R = mybir.dt.float32r
AT = mybir.ActivationFunctionType
ALU = mybir.AluOpType

B, N, E, H, NLAYERS = 2, 16384, 8, 256, 4
OMEGA0 = 30.0
NCORES = 8
PTS = B * N // NCORES            # 4096 points per core
CHUNK = 1024
NCHUNK = PTS // CHUNK            # 4
SUB = 512                        # matmul moving free dim
NSUB = CHUNK // SUB              # 4 subtiles per chunk
ZGRP = 1024                      # DVE wrap granularity (2 PSUM banks)
NZG = CHUNK // ZGRP              # 2 z-groups per chunk
NHID = NLAYERS - 1               # 3 hidden layers
TWO_PI = float(2.0 * np.pi)
SC = float(OMEGA0 / (2.0 * np.pi))   # pre-scale: radians -> turns
MAGIC = float(np.float32(1.5 * 2 ** 23))
OUTCOLS = PTS // 128             # 32 columns of transposed output per core

# consts tile column layout ([128, 256] fp32)
C_A0 = 0       # 16 cols: layer0 scale  (e*2+half)
C_C0 = 16      # 16 cols: layer0 bias
C_WO = 80      # 16 cols: output weights (e*2+kc), lhsT column [128,1]
C_GB = 96      # 1 col: gate bias (partitions 0..7)
C_BO = 97      # 1 col: output bias (partitions 0..7)
C_ONES8 = 112  # 1 col: ones (partitions 0..7)
C_GW = 104     # row 0, cols 104..111: gate weights (lhsT [1,8])
C_ONES1 = 128  # row 0, cols 128..255: ones (lhsT [1,128])
C_CH = 32      # 48 cols: hidden bias ((l-1)*16 + e*2 + half)
C_WO8 = 256    # 128 cols: zero-padded output lhsT blocks [(e*2+kc)*8 + e']

_BUILD_CACHE: dict = {}


def _build(wrap_twice: bool, sin_units: int = 1, v_bufs: int = 1,
           l0_bufs: int = 3, dve_cols=(1, 3, 5, 7, 9, 11, 13, 15), z_bufs: int = 3):
    nc = bacc.Bacc("TRN2", target_bir_lowering=False, debug=False,
                   num_devices=NCORES)

    d_x = nc.dram_tensor("x", [1, PTS], F32, kind="ExternalInput")
    d_wh = nc.dram_tensor("wh", [128, NHID * 4096], F32, kind="ExternalInput")
    d_consts = nc.dram_tensor("consts", [128, 384], F32, kind="ExternalInput")
    d_out = nc.dram_tensor("out", [128, OUTCOLS], F32, kind="ExternalOutput")

    UW = CHUNK                       # unit width (one (m, half) slab)
    NB = 8 * UW                      # big-tile width: 8 units

    with tile.TileContext(nc) as tc:
        with (
            tc.tile_pool(name="cst", bufs=1) as cst_pool,
            tc.tile_pool(name="whp", bufs=1) as wh_pool,
            tc.tile_pool(name="io", bufs=1) as io_pool,
            tc.tile_pool(name="hbuf", bufs=1) as h_pool,
            tc.tile_pool(name="vbuf", bufs=1) as v_pool,
            tc.tile_pool(name="tmp", bufs=1) as tmp_pool,
            tc.tile_pool(name="zps", bufs=1, space="PSUM") as z_ps,
            tc.tile_pool(name="yps", bufs=1, space="PSUM") as y_ps,
        ):
            t_cst = cst_pool.tile([128, 384], F32, tag="consts")
            nc.sync.dma_start(t_cst[:], d_consts[:, :])
            ap_gb = t_cst[0:8, C_GB:C_GB + 1]
            ap_bo = t_cst[0:8, C_BO:C_BO + 1]
            ap_ones8 = t_cst[0:8, C_ONES8:C_ONES8 + 1]
            ap_gw = t_cst[0:1, C_GW:C_GW + 8]
            ap_ones1 = t_cst[0:1, C_ONES1:C_ONES1 + 128]

            # hidden + output weights, rounded to fp32r via casting DMA
            t_wh = []
            for l in range(NHID):
                w = wh_pool.tile([128, 4096], F32R, tag=f"wh{l}", name=f"wh{l}")
                nc.gpsimd.dma_start(w[:], d_wh[:, l * 4096:(l + 1) * 4096])
                t_wh.append(w)
            t_wo8 = wh_pool.tile([128, 128], F32R, tag="wo8", name="wo8")
            nc.gpsimd.dma_start(t_wo8[:], d_consts[:, C_WO8:C_WO8 + 128])

            t_x = io_pool.tile([1, PTS], F32, tag="x")
            nc.sync.dma_start(t_x[:], d_x[0:1, :])

            # ---- gate preamble over all points: u = exp(gw*x+gb); rsT = 1/sum
            t_u = io_pool.tile([8, PTS], F32, tag="u")
            for s in range(PTS // SUB):
                p_zg = y_ps.tile([8, SUB], F32, tag="y", name=f"zg{s}", bufs=2)
                nc.tensor.matmul(p_zg[:], ap_gw, t_x[:, s * SUB:(s + 1) * SUB],
                                 start=True, stop=True)
                nc.scalar.activation(t_u[:, s * SUB:(s + 1) * SUB], p_zg[:],
                                     AT.Exp, bias=ap_gb, scale=1.0)
            p_den = z_ps.tile([128, ZGRP], F32, tag="z", name="pden",
                              bufs=z_bufs)
            for col in range(OUTCOLS):
                nc.tensor.matmul(p_den[:, col:col + 1],
                                 t_u[:, col * 128:(col + 1) * 128],
                                 ap_ones8, start=True, stop=True)
            t_rso = tmp_pool.tile([128, 2 * OUTCOLS], F32, tag="rso")
            nc.vector.reciprocal(t_rso[:, 0:OUTCOLS], p_den[:, 0:OUTCOLS])

            # x broadcast tiles, pipelined one chunk ahead
            t_xb = {}

            def emit_xb(c):
                t = io_pool.tile([128, CHUNK], F32, tag="xb", bufs=2,
                                 name=f"xb{c}")
                for s in range(NSUB):
                    g = c * NSUB + s
                    p_xb = z_ps.tile([128, ZGRP], F32, tag="z",
                                     name=f"pxb{c}_{s}", bufs=z_bufs)
                    nc.tensor.matmul(p_xb[:, 0:SUB], ap_ones1,
                                     t_x[:, g * SUB:(g + 1) * SUB],
                                     start=True, stop=True)
                    nc.vector.tensor_copy(t[:, s * SUB:(s + 1) * SUB],
                                          p_xb[:, 0:SUB])
                t_xb[c] = t

            emit_xb(0)

            for c in range(NCHUNK):
                if c + 1 < NCHUNK:
                    emit_xb(c + 1)

                chunk_y = [y_ps.tile([8, SUB], F32, tag="y", name=f"y{c}_{s}",
                                     bufs=2)
                           for s in range(NSUB)]

                for quad in range(2):
                    # big per-parity activation slabs: unit = m*2 + half
                    t_h = {par: h_pool.tile([128, NB], F32R, tag=f"hb{par}",
                                            name=f"hb{c}_{quad}_{par}")
                           for par in range(2)}

                    # layer 0 (affine + magic-round frac), Pool + some DVE,
                    # fully decoupled via its own double-buffered tiles so it
                    # can run ahead of the hidden-layer pipeline
                    for m in range(4):
                        e = quad * 4 + m
                        for half in range(2):
                            col = e * 2 + half
                            unit = m * 2 + half
                            eng = nc.vector if col in dve_cols else nc.gpsimd
                            t_zb = tmp_pool.tile([128, UW], F32,
                                                 tag="zb", bufs=l0_bufs,
                                                 name=f"zb{c}{quad}{col}")
                            eng.tensor_scalar(
                                t_zb[:], t_xb[c][:],
                                t_cst[:, C_A0 + col:C_A0 + col + 1],
                                t_cst[:, C_C0 + col:C_C0 + col + 1],
                                ALU.mult, ALU.add)
                            t_k = tmp_pool.tile([128, UW], F32,
                                                tag="k", bufs=l0_bufs,
                                                name=f"k{c}{quad}{col}")
                            eng.tensor_scalar(t_k[:], t_zb[:],
                                              MAGIC, MAGIC,
                                              ALU.add, ALU.subtract)
                            t_v0 = tmp_pool.tile([128, UW], F32,
                                                 tag="v0", bufs=l0_bufs,
                                                 name=f"v0_{c}{quad}{col}")
                            eng.tensor_tensor(t_v0[:], t_zb[:], t_k[:],
                                              ALU.subtract)
                            nc.scalar.activation(
                                t_h[0][:, unit * UW:(unit + 1) * UW],
                                t_v0[:], AT.Sin, bias=0.0, scale=TWO_PI)

                    # hidden layers, 4-expert staggered
                    for l in range(1, NLAYERS):
                        lw = l - 1
                        rpar = (l - 1) & 1
                        wpar = l & 1
                        for m in range(4):
                            e = quad * 4 + m
                            for half in range(2):
                                unit = m * 2 + half
                                for g in range(NZG):
                                    p_z = z_ps.tile([128, ZGRP], F32, tag="z",
                                                    name=f"z{m}{half}{g}",
                                                    bufs=z_bufs)
                                    for si in range(ZGRP // SUB):
                                        s = g * (ZGRP // SUB) + si
                                        for kc in range(2):
                                            wc = ((e * 2 + kc) * 2 + half) * 128
                                            ru = m * 2 + kc
                                            nc.tensor.matmul(
                                                p_z[:, si * SUB:(si + 1) * SUB],
                                                t_wh[lw][:, wc:wc + 128],
                                                t_h[rpar][:, ru * UW + s * SUB:
                                                           ru * UW + (s + 1) * SUB],
                                                start=(kc == 0), stop=(kc == 1))
                                    chc = C_CH + lw * 16 + e * 2 + half
                                    t_v = v_pool.tile(
                                        [128, ZGRP], F32, tag="vh",
                                        name=f"vh{c}{quad}{m}{half}", bufs=3)
                                    vsl = slice(0, ZGRP)
                                    if wrap_twice:
                                        t_t2 = tmp_pool.tile([128, ZGRP], F32,
                                                             tag="wr2")
                                        nc.vector._custom_dve(
                                            ADD_RANGE_WRAP, out=t_t2[:],
                                            in0=p_z[:],
                                            s0=t_cst[:, chc:chc + 1],
                                            s1=1.0, imm2=2.0)
                                        nc.vector._custom_dve(
                                            ADD_RANGE_WRAP,
                                            out=t_v[:, vsl],
                                            in0=t_t2[:], s0=0.0,
                                            s1=0.5, imm2=1.0)
                                    else:
                                        nc.vector._custom_dve(
                                            ADD_RANGE_WRAP,
                                            out=t_v[:, vsl],
                                            in0=p_z[:],
                                            s0=t_cst[:, chc:chc + 1],
                                            s1=0.5, imm2=1.0)
                                    nc.scalar.activation(
                                        t_h[wpar][:, unit * UW + g * ZGRP:
                                                  unit * UW + (g + 1) * ZGRP],
                                        t_v[:], AT.Sin, bias=0.0,
                                        scale=TWO_PI)

                    # output layer: long accumulation group per subtile;
                    # zero-padded M=8 lhsT adds only row e per matmul
                    for s in range(NSUB):
                        p_y = chunk_y[s]
                        for m in range(4):
                            e = quad * 4 + m
                            for kc in range(2):
                                ru = m * 2 + kc
                                blk = (e * 2 + kc) * 8
                                nc.tensor.matmul(
                                    p_y[:, :],
                                    t_wo8[:, blk:blk + 8],
                                    t_h[1][:, ru * UW + s * SUB:
                                           ru * UW + (s + 1) * SUB],
                                    start=(quad == 0 and m == 0 and kc == 0),
                                    stop=(quad == 1 and m == 3 and kc == 1),
                                    skip_group_check=True)

                # ---- combine
                t_w8 = io_pool.tile([8, CHUNK], F32, tag="w8")
                for s in range(NSUB):
                    nc.vector.scalar_tensor_tensor(
                        t_w8[:, s * SUB:(s + 1) * SUB], chunk_y[s][:], ap_bo,
                        t_u[:, (c * NSUB + s) * SUB:(c * NSUB + s + 1) * SUB],
                        ALU.add, ALU.mult)
                nco = CHUNK // 128
                p_num = z_ps.tile([128, ZGRP], F32, tag="z", name=f"pnum{c}",
                                  bufs=z_bufs)
                for col in range(nco):
                    nc.tensor.matmul(p_num[:, col:col + 1],
                                     t_w8[:, col * 128:(col + 1) * 128],
                                     ap_ones8, start=True, stop=True)
                nc.vector.tensor_tensor(
                    t_rso[:, OUTCOLS + c * nco:OUTCOLS + (c + 1) * nco],
                    p_num[:, 0:nco],
                    t_rso[:, c * nco:(c + 1) * nco], ALU.mult)

            nc.sync.dma_start(d_out[:, :], t_rso[:, OUTCOLS:2 * OUTCOLS])

    nc.compile()
    return nc


LAST_RESULT = None  # BassKernelResults of the most recent run (for test.py)


def kernel(x, gate_w, gate_b, w0, b0, wh, bh, wo, bo):
    x = np.asarray(x, dtype=np.float32)
    gate_w = np.asarray(gate_w, dtype=np.float32)
    gate_b = np.asarray(gate_b, dtype=np.float32)
    w0 = np.asarray(w0, dtype=np.float32)
    b0 = np.asarray(b0, dtype=np.float32)
    wh = np.asarray(wh, dtype=np.float32)
    bh = np.asarray(bh, dtype=np.float32)
    wo = np.asarray(wo, dtype=np.float32)
    bo = np.asarray(bo, dtype=np.float32)

    # Hidden pre-activation range (in turns) decides single vs double wrap.
    # The static L1 bound is far too pessimistic; measure on the actual data
    # with a host forward pass (batched to bound memory).
    hid_bound = 0.0
    xf0 = x.reshape(-1)
    for lo in range(0, xf0.size, 8192):
        xs = xf0[lo:lo + 8192]
        h = np.sin(OMEGA0 * (w0[:, :, 0:1] * xs[None, None, :]
                             + b0[:, :, None])).astype(np.float32)
        for l in range(NHID):
            z = SC * (np.einsum('egh,eht->egt', wh[l], h,
                                dtype=np.float32)
                      + bh[l][:, :, None]).astype(np.float32)
            hid_bound = max(hid_bound, float(np.abs(z).max()))
            h = np.sin(TWO_PI * z).astype(np.float32)
    hid_bound *= 1.02  # slack for HW fp divergence
    assert hid_bound < 2.90, f"hidden range {hid_bound} too large for 2 wraps"
    wrap_twice = hid_bound >= 1.45

    # ---- host packing (fp32)
    whp = np.zeros((128, NHID * 4096), dtype=np.float32)
    for l in range(NHID):
        for e in range(E):
            for kc in range(2):
                for mc in range(2):
                    colbase = l * 4096 + ((e * 2 + kc) * 2 + mc) * 128
                    blk = (SC * wh[l, e, mc * 128:(mc + 1) * 128,
                                   kc * 128:(kc + 1) * 128]).T  # [k, m]
                    whp[:, colbase:colbase + 128] = blk

    consts = np.zeros((128, 384), dtype=np.float32)
    for e in range(E):
        for half in range(2):
            col = e * 2 + half
            consts[:, C_A0 + col] = SC * w0[e, half * 128:(half + 1) * 128, 0]
            consts[:, C_C0 + col] = SC * b0[e, half * 128:(half + 1) * 128]
    for l in range(NHID):
        for e in range(E):
            for half in range(2):
                consts[:, C_CH + l * 16 + e * 2 + half] = \
                    SC * bh[l, e, half * 128:(half + 1) * 128]
    for e in range(E):
        for kc in range(2):
            consts[:, C_WO8 + (e * 2 + kc) * 8 + e] = \
                wo[e, 0, kc * 128:(kc + 1) * 128]
    consts[0:8, C_GB] = gate_b
    consts[0:8, C_BO] = bo[:, 0]
    consts[0:8, C_ONES8] = 1.0
    consts[0, C_GW:C_GW + 8] = gate_w[:, 0]
    consts[0, C_ONES1:C_ONES1 + 128] = 1.0

    xf = x.reshape(-1)                      # [B*N]
    in_maps = []
    for j in range(NCORES):
        xc = xf[j * PTS:(j + 1) * PTS].reshape(1, PTS).copy()
        in_maps.append({"x": xc, "wh": whp, "consts": consts})

    key = wrap_twice
    if key not in _BUILD_CACHE:
        _BUILD_CACHE[key] = _build(wrap_twice)
    nc = _BUILD_CACHE[key]

    global LAST_RESULT
    LAST_RESULT = run_bass_kernel_spmd(nc, in_maps, list(range(NCORES)))
    res = LAST_RESULT.results
    parts = []
    for j in range(NCORES):
        outT = res[j]["out"]                # [128, 32]
        parts.append(outT.T.reshape(-1))    # point t = col*128 + p
    full = np.concatenate(parts).astype(np.float32)
    return full.reshape(B, N, 1)

